# revision 38
# baseline (speedup 1.0000x reference)
"""GSMNet GNN message-passing layer on 8 Trainium2 NeuronCores.

Fused single-pass design:
  * Edges are partitioned across cores BY DESTINATION NODE (core c owns dst
    nodes [c*N/8, (c+1)*N/8)), each core's edges sorted by destination, so
    the per-node aggregation is core-local.  Scatter-add is done with one-hot
    matmuls into a sliding node window with static per-tile base offsets.
  * The host pre-computes everything cheap in edge/node space: the 3-neighbor
    sums of edge_nei_len/angle, the per-node transforms x@Wf1[a,b] / x@Wm1[a,b]
    gathered+summed per edge (qf, qm), the envelope cos^2 weights, the one-hot
    scatter masks, and packs all per-edge operands FEATURE-MAJOR so the device
    does no transposes and no downcasts on the input path.
  * The edge-update MLP (u1/gate/update/ef_lin) runs in fp8e4 DoubleRow
    matmuls (2x PE throughput, K=256 per instruction); its noise is washed
    through LayerNorm + the downstream squashing nonlinearities.  The message
    MLPs (direct output path) stay bf16.
  * BatchNorm-intermediate statistics (used inside a sigmoid only) are
    estimated on the host from a 32k-edge sample, removing the two-phase
    structure, the z/msg DRAM scratch round-trip and one AllReduce entirely.
    BatchNorm-out stats stay exact (single [128,4] AllReduce at the end).
  * LayerNorm row statistics are transposed into a [128,4] column form so the
    mean/var/rsqrt math runs 128-wide; rsqrt is a quake-style bit-trick + two
    Newton steps on the DVE, keeping the ACT engine to Silu/Sigmoid/Relu/Copy
    (no activation-table thrashing).
  * A 3-stage software pipeline (A: edge-update matmuls, B: LN scalar chain,
    C: message MLPs + scatter) keeps the PE fed while LN latency percolates.
"""

import math

import ml_dtypes
import numpy as np

import bass_rust
import concourse.bass as bass
import concourse.mybir as mybir
import concourse.tile as tile
from concourse.bass_utils import run_bass_kernel_spmd
from concourse.vector_clock import ScopedClock

dt = mybir.dt
F32 = dt.float32
BF16 = dt.bfloat16
FP8 = dt.float8e4
U32 = dt.uint32
NBF = ml_dtypes.bfloat16
NF8 = ml_dtypes.float8_e4m3
ALU = mybir.AluOpType
ACTF = mybir.ActivationFunctionType
DR = mybir.MatmulPerfMode.DoubleRow

NCORES = 8
H = 256
ETILE = 512
CUTOFF = 5.0
NSAMP = 32768   # edges sampled for host-side BN-int stats
WSCALE = 16.0   # fp8 weight pre-scale (power of 2)
FP8_EDGE = True


# ---------------------------------------------------------------------------
# Walrus in this container rejects instructions carrying several semaphore
# waits on the no-struct ctrl path (the TileContext tail drain).  Split the
# drain's waits across single-wait nops.
_PATCHED = False


def _patch_tile_drain():
    global _PATCHED
    if _PATCHED:
        return

    _orig_lower = tile.TileContext._lower_ordered_insts
    _skip_types = ("TileBranchInst", "BassTileLoopBlock")
    _ws_id = [0]

    def _split_lower(self, ordered):
        for bb_name, insts in list(ordered.items()):
            new = []
            for inst in insts:
                if type(inst).__name__ in _skip_types:
                    new.append(inst)
                    continue
                try:
                    si = inst.sync_info
                    waits = list(si.on_wait) if si is not None else []
                except Exception:
                    waits = []
                if len(waits) > 1:
                    for w in waits[:-1]:
                        ev = bass_rust.InstEventSemaphore(
                            name=f"WS-{_ws_id[0]}")
                        _ws_id[0] += 1
                        ev.engine = inst.engine
                        ev.sync_info = bass_rust.SyncInfo(
                            on_wait=[w], on_update=[])
                        new.append(ev)
                    inst.sync_info = bass_rust.SyncInfo(
                        on_wait=[waits[-1]], on_update=list(si.on_update))
                new.append(inst)
            ordered[bb_name] = new
        return _orig_lower(self, ordered)

    tile.TileContext._lower_ordered_insts = _split_lower

    def _drain_and_barrier(self, tick_clock, wait_clock):
        probe = self.nc.sync.nop(nofuse=True)
        wait_clock.add_sem_waits(
            probe.ins, ScopedClock({None: tick_clock.global_clock})
        )
        waits = list(probe.ins.sync_info.on_wait)
        probe.ins.sync_info = bass_rust.SyncInfo(on_wait=waits[:1], on_update=[])
        for w in waits[1:]:
            inst = self.nc.sync.nop(nofuse=True)
            inst.ins.sync_info = bass_rust.SyncInfo(on_wait=[w], on_update=[])
        self.nc.sync.drain()
        self.nc.all_engine_barrier()
        popped = self.nc._tile_sem_poison_stack.pop()
        assert popped is self._sem_poison
        self.nc.clear_and_free_semaphores(list(self.sems.allocated().values()))
        self.nc.all_engine_barrier()

    tile.TileContext._drain_and_barrier = _drain_and_barrier
    _PATCHED = True


# ---------------------------------------------------------------------------
# host-side numerics helpers

FP8_WEIGHTS = {"u1f", "u1l", "u1a", "we", "w2", "gf", "gu"} if FP8_EDGE else set()
WEIGHT_NAMES = ["u1f", "u1l", "u1a", "we", "w2", "gf", "gu",
                "f1c", "m1c", "f2", "m2"]
BIAS_ORDER = ["u1b", "be", "b2", "gb", "lng", "lnb", "bf1", "bm1",
              "As", "Bs", "bm2", "bnog", "bnob"]


def _bfr(a):
    # bf16 round-trip (matches device operand rounding)
    return np.asarray(a, np.float32).astype(NBF).astype(np.float32)


def _f8r(a):
    # fp8e4m3 round-trip
    return np.asarray(a, np.float32).astype(NF8).astype(np.float32)


def _pack_w(w, fp8):
    # [K, M] -> [128, K//128, M] lhsT-chunk layout
    K, M = w.shape
    assert K % 128 == 0
    p = np.ascontiguousarray(w.reshape(K // 128, 128, M).transpose(1, 0, 2))
    if fp8:
        return (p * WSCALE).astype(NF8)
    return p.astype(NBF)


def _pack_b(b):
    # [256] -> [128, 2] per-partition chunks, fp32
    return np.ascontiguousarray(np.asarray(b).reshape(2, 128).T).astype(np.float32)


def _cols(a, NT):
    # [E_pad] -> [128, NT*4]: edge (t,s,p) at [p, t*4+s]
    return np.ascontiguousarray(
        np.asarray(a, np.float32).reshape(NT * 4, 128).T
    )


def _featmajor(a, NT, npdt):
    # [E_pad, 256] -> [128, NT, 2, 512]: [p, t, c, e] = a[t*512+e, c*128+p]
    E_pad = a.shape[0]
    assert E_pad == NT * ETILE
    return np.ascontiguousarray(
        np.asarray(a, np.float32).reshape(NT, ETILE, 2, 128).transpose(3, 0, 2, 1)
    ).astype(npdt)


def _fold_weights(ins):
    g = lambda k: np.asarray(ins[k], np.float64)
    We, be = g("eu_lin_edge_w"), g("eu_lin_edge_b")
    Wl, bl = g("eu_lin_len_w"), g("eu_lin_len_b")
    Wa, ba = g("eu_lin_ang_w"), g("eu_lin_ang_b")
    W1, b1 = g("eu_up1_w"), g("eu_up1_b")
    W2, b2 = g("eu_up2_w"), g("eu_up2_b")
    Wg, bg = g("eu_gate_w"), g("eu_gate_b")
    Wf1, bf1 = g("mp_full1_w"), g("mp_full1_b")
    Wf2, bf2 = g("mp_full2_w"), g("mp_full2_b")
    Wm1, bm1 = g("mp_msg1_w"), g("mp_msg1_b")
    Wm2, bm2 = g("mp_msg2_w"), g("mp_msg2_b")

    W1a, W1b, W1c = W1[0:H], W1[H:2 * H], W1[2 * H:3 * H]
    Wga, Wgb = Wg[0:H], Wg[H:2 * H]
    weights = {
        "u1f": We @ W1a,
        "u1l": (Wl @ W1b) / 3.0,
        "u1a": (Wa @ W1c) / 3.0,
        "we": We,
        "w2": W2,
        "gf": We @ Wga,
        "gu": W2 @ Wgb,
        "f1a": Wf1[0:H], "f1b": Wf1[H:2 * H], "f1c": Wf1[2 * H:3 * H],
        "f2": Wf2,
        "m1a": Wm1[0:H], "m1b": Wm1[H:2 * H], "m1c": Wm1[2 * H:3 * H],
        "m2": Wm2,
    }
    biases = {
        "u1b": b1 + be @ W1a + bl @ W1b + ba @ W1c,
        "be": be, "b2": b2,
        "gb": bg + be @ Wga + b2 @ Wgb,
        "lng": g("eu_ln_g"), "lnb": g("eu_ln_b"),
        "bf1": bf1, "bf2": bf2, "bm1": bm1, "bm2": bm2,
        "bnig": g("bn_int_g"), "bnib": g("bn_int_b"),
        "bnog": g("bn_out_g"), "bnob": g("bn_out_b"),
    }
    return weights, biases


def _silu(v):
    return v / (1.0 + np.exp(-v))


def _rnd_edge(w):
    # host mirror of how the device rounds an edge-update operand
    if FP8_EDGE:
        return _f8r(np.asarray(w, np.float64) * WSCALE) / WSCALE
    return _bfr(w)


def _sample_bn_int_stats(weights, biases, ef, sl, sa, qf):
    """Mirror the device pipeline on a sample of edges; return (mean, var)
    of z over the sample."""
    we = {k: _rnd_edge(weights[k]) for k in
          ("u1f", "u1l", "u1a", "we", "w2", "gf", "gu")}
    wm = {k: _bfr(weights[k]) for k in ("f1c", "f2")}
    b = {k: np.asarray(biases[k], np.float32) for k in biases}
    rnd_in = _f8r if FP8_EDGE else _bfr
    efr, slr, sar = rnd_in(ef), rnd_in(sl), rnd_in(sa)
    u1 = efr @ we["u1f"] + slr @ we["u1l"] + sar @ we["u1a"] + b["u1b"]
    u1s = rnd_in(_silu(u1))
    gate = _bfr(1.0 / (1.0 + np.exp(-(efr @ we["gf"] + u1s @ we["gu"] + b["gb"]))))
    upd = _bfr(u1s @ we["w2"] + b["b2"])
    efc = _bfr(efr @ we["we"] + b["be"])
    y = _bfr(_bfr(gate * upd) + efc)
    m = y.mean(-1, keepdims=True)
    v = y.var(-1, keepdims=True)
    inv = _bfr(1.0 / np.sqrt(v + 1e-5))
    n = _bfr(m * inv)
    e = _bfr(_bfr(y * inv) - n)
    eo = _bfr(np.maximum(e * b["lng"] + b["lnb"], 0.0))
    h1 = _bfr(_silu(_bfr(qf) + eo @ wm["f1c"] + b["bf1"]))
    z = h1 @ wm["f2"] + b["bf2"]
    return z.mean(0), z.var(0)


def _prepare(inputs):
    x = np.asarray(inputs["x"], np.float32)
    ei = np.asarray(inputs["edge_index"])
    ef = np.asarray(inputs["edge_features"], np.float32)
    enl = np.asarray(inputs["edge_nei_len"], np.float32)
    ena = np.asarray(inputs["edge_nei_angle"], np.float32)
    el = np.asarray(inputs["edge_length"], np.float32)

    N, Hx = x.shape
    assert Hx == H
    E = ef.shape[0]
    assert N % NCORES == 0
    NLOC = N // NCORES
    sl = enl.sum(1)
    sa = ena.sum(1)

    src = np.asarray(ei[0], np.int64)
    dst = np.asarray(ei[1], np.int64)

    weights, biases = _fold_weights(inputs)

    # per-node transforms, gathered per edge
    w32 = lambda k: _bfr(weights[k])
    qf = (x @ w32("f1a"))[dst] + (x @ w32("f1b"))[src]
    qm = (x @ w32("m1a"))[dst] + (x @ w32("m1b"))[src]

    # host-side BN-int statistics from an edge sample
    step = max(1, E // NSAMP)
    sel = np.arange(0, E, step)
    mu, var = _sample_bn_int_stats(
        weights, biases, ef[sel], sl[sel], sa[sel], qf[sel])
    As = np.asarray(biases["bnig"], np.float64) / np.sqrt(var + 1e-5)
    Bs = np.asarray(biases["bnib"], np.float64) - mu * As
    biases["As"] = As
    biases["Bs"] = Bs + As * np.asarray(biases["bf2"], np.float64)
    # fold the rsqrt-by-bit-trick x2 gamma convention: none needed (exact)

    env = np.where(el < CUTOFF,
                   np.cos(el * (math.pi / (2.0 * CUTOFF))) ** 2,
                   0.0).astype(np.float32)

    core_of = dst // NLOC
    perms, counts = [], []
    for c in range(NCORES):
        ids = np.nonzero(core_of == c)[0]
        order = np.argsort(dst[ids], kind="stable")
        perms.append(ids[order])
        counts.append(len(ids))
    NT = max(1, -(-max(counts) // ETILE))
    E_pad = NT * ETILE

    # static per-tile scatter-window bases shared across cores
    INF = 1 << 30
    lo = np.full((NCORES, NT), INF, np.int64)
    hi = np.full((NCORES, NT), -1, np.int64)
    for c in range(NCORES):
        dl = dst[perms[c]] - c * NLOC
        for t in range(NT):
            seg = dl[t * ETILE:(t + 1) * ETILE]
            if len(seg):
                lo[c, t] = seg[0]
                hi[c, t] = seg[-1]
    lo_t = lo.min(axis=0)
    hi_t = hi.max(axis=0)
    W = 128
    while True:
        base = np.minimum(np.where(lo_t == INF, 0, lo_t), max(NLOC - W, 0))
        if np.all(hi_t < base + W):
            break
        if W >= min(512, NLOC):
            raise RuntimeError("scatter window overflow")
        W = min(W * 2, 512, NLOC)
    base = base.astype(np.int64)

    wmaps = {f"w_{k}": _pack_w(np.asarray(weights[k], np.float64),
                               k in FP8_WEIGHTS)
             for k in WEIGHT_NAMES}
    bias_arr = np.concatenate([_pack_b(np.asarray(biases[k], np.float32))
                               for k in BIAS_ORDER], axis=1)
    identb = np.eye(128, dtype=np.float32).astype(NBF)
    identf = np.eye(128, dtype=np.float32)

    edge_npdt = NF8 if FP8_EDGE else NBF

    in_maps = []
    for c in range(NCORES):
        p = perms[c]
        cnt = counts[c]

        def padded(a):
            out = np.zeros((E_pad, H), np.float32)
            out[:cnt] = a[p]
            return out

        env_p = np.zeros(E_pad, np.float32)
        env_p[:cnt] = env[p]
        dl = np.zeros(E_pad, np.int64)
        dl[:cnt] = dst[p] - c * NLOC
        tile_of = np.arange(E_pad) // ETILE
        drel = dl - base[tile_of]
        drel[cnt:] = 0
        assert drel.min() >= 0 and drel.max() < W

        # host one-hot scatter masks scaled by the envelope:
        # [128, NT, 4, W], edge (t,s,p) row
        ohm = np.zeros((E_pad, W), np.float32)
        ohm[np.arange(E_pad), drel] = env_p
        ohm[cnt:] = 0.0
        ohm = np.ascontiguousarray(
            ohm.reshape(NT, 4, 128, W).transpose(2, 0, 1, 3)).astype(NBF)

        in8 = np.stack([_featmajor(padded(a), NT, edge_npdt)
                        for a in (ef, sl, sa)], axis=2)
        in16 = np.stack([_featmajor(padded(a), NT, NBF)
                         for a in (qf, qm)], axis=2)

        m = {
            "in8": np.ascontiguousarray(in8),
            "in16": np.ascontiguousarray(in16),
            "inoh": ohm,
            "biases": bias_arr.astype(np.float32),
            "identb": identb,
            "identf": identf,
            "xT_loc": np.ascontiguousarray(x[c * NLOC:(c + 1) * NLOC].T),
        }
        m.update(wmaps)
        in_maps.append(m)

    cfg = dict(N=N, NLOC=NLOC, E=E, E_pad=E_pad, NT=NT, W=W,
               base=tuple(int(b) for b in base))
    return cfg, in_maps


# ---------------------------------------------------------------------------
# device program


def _build_program(cfg):
    _patch_tile_drain()
    N, NLOC, E_pad, NT, W = cfg["N"], cfg["NLOC"], cfg["E_pad"], cfg["NT"], cfg["W"]
    base = cfg["base"]
    EDT = FP8 if FP8_EDGE else BF16
    IS = 1.0 / WSCALE if FP8_EDGE else 1.0

    nc = bass.Bass("TRN2", target_bir_lowering=False, debug=False,
                   num_devices=NCORES)

    in8_d = nc.dram_tensor("in8", [128, NT, 3, 2, ETILE], EDT,
                           kind="ExternalInput")
    in16_d = nc.dram_tensor("in16", [128, NT, 2, 2, ETILE], BF16,
                            kind="ExternalInput")
    inoh_d = nc.dram_tensor("inoh", [128, NT, 4, W], BF16,
                            kind="ExternalInput")
    bias_d = nc.dram_tensor("biases", [128, 2 * len(BIAS_ORDER)], F32,
                            kind="ExternalInput")
    identb_d = nc.dram_tensor("identb", [128, 128], BF16, kind="ExternalInput")
    identf_d = nc.dram_tensor("identf", [128, 128], F32, kind="ExternalInput")
    xT_d = nc.dram_tensor("xT_loc", [H, NLOC], F32, kind="ExternalInput")
    w_d = {k: nc.dram_tensor(f"w_{k}", [128, 2, H],
                             FP8 if k in FP8_WEIGHTS else BF16,
                             kind="ExternalInput")
           for k in WEIGHT_NAMES}

    out_d = nc.dram_tensor("out", [H, NLOC], F32, kind="ExternalOutput")

    ccB_in = nc.dram_tensor("ccB_in", [128, 4], F32)
    ccB_out = nc.dram_tensor("ccB_out", [128, 4], F32, addr_space="Shared")
    RG = [list(range(NCORES))]

    with tile.TileContext(nc) as tc:
        with (
            tc.tile_pool(name="const", bufs=1) as cp,
            tc.tile_pool(name="io", bufs=4) as io,
            tc.tile_pool(name="wk", bufs=2) as wk,
            tc.tile_pool(name="ps", bufs=1, space="PSUM") as ps,
        ):
            # ---- resident constants
            wt = {}

            def _load_w(k):
                t = cp.tile([128, 2, H], FP8 if k in FP8_WEIGHTS else BF16,
                            name=f"wt_{k}")
                nc.sync.dma_start(t[:], w_d[k][:])
                wt[k] = t

            for k in ("u1f", "u1l", "u1a"):
                _load_w(k)
            bias_t = cp.tile([128, 2 * len(BIAS_ORDER)], F32)
            nc.sync.dma_start(bias_t[:], bias_d[:])
            # preload the first tiles' inputs ahead of the remaining weights
            st = {}

            def load_tile(t):
                s = st.setdefault(t, {})
                in8b = s["in8"] = io.tile([128, 3, 2, ETILE], EDT, tag="in8",
                                          name=f"in8_{t}")
                nc.sync.dma_start(in8b[:], in8_d[:, t])
                in16b = s["in16"] = io.tile([128, 2, 2, ETILE], BF16,
                                            tag="in16", name=f"in16_{t}",
                                            bufs=4)
                nc.sync.dma_start(in16b[:], in16_d[:, t])
                ohb = s["oh"] = io.tile([128, 4, W], BF16, tag="oh",
                                        name=f"oh{t}", bufs=4)
                nc.sync.dma_start(ohb[:], inoh_d[:, t])

            for _pt in range(min(2, NT)):
                load_tile(_pt)
            for k in WEIGHT_NAMES:
                if k not in wt:
                    _load_w(k)

            def B(name, mc):
                i = BIAS_ORDER.index(name)
                return bias_t[:, 2 * i + mc: 2 * i + mc + 1]

            identb_t = cp.tile([128, 128], BF16)
            nc.sync.dma_start(identb_t[:], identb_d[:])
            identf_t = cp.tile([128, 128], F32)
            nc.sync.dma_start(identf_t[:], identf_d[:])
            xT_t = cp.tile([128, 2, NLOC], F32)
            ones_cb = cp.tile([128, 1], BF16)
            nc.vector.memset(ones_cb[:], 1.0)
            ones_rb = cp.tile([1, 128], BF16)
            nc.vector.memset(ones_rb[:], 1.0)
            magic_t = cp.tile([128, 4], U32)
            nc.vector._memset_packed(magic_t[:], 0x5F3759DF)

            agg = [cp.tile([128, NLOC], F32, name=f"agg{c}") for c in range(2)]
            nc.vector.memset(agg[0][:], 0.0)
            nc.vector.memset(agg[1][:], 0.0)

            def mm(psum, pairs, tail=None):
                n = len(pairs) + (1 if tail else 0)
                for i, (w, kc, mc, rhs) in enumerate(pairs):
                    nc.tensor.matmul(
                        psum[:], wt[w][:, kc, mc * 128:(mc + 1) * 128],
                        rhs, start=(i == 0), stop=(i == n - 1))
                if tail:
                    nc.tensor.matmul(psum[:], tail[0], tail[1],
                                     start=False, stop=True)

            def mm_edge(psum, triples):
                # edge-update matmuls: fp8 DoubleRow (K=256/instr) or bf16
                if FP8_EDGE:
                    for i, (w, mc, rhs) in enumerate(triples):
                        nc.tensor.matmul(
                            psum[:], wt[w][:, :, mc * 128:(mc + 1) * 128],
                            rhs, start=(i == 0), stop=(i == len(triples) - 1),
                            perf_mode=DR)
                else:
                    pairs = [(w, kc, mc, rhs[:, kc])
                             for (w, mc, rhs) in triples for kc in range(2)]
                    mm(psum, pairs)

            # -------- 3-stage software pipeline over edge tiles.

            def stageA1(t):
                if t not in st:
                    load_tile(t)
                s = st[t]
                in8b = s["in8"]
                fT, lT, aT = in8b[:, 0], in8b[:, 1], in8b[:, 2]

                # u1 = silu(ef@U1f + sl@U1l + sa@U1a + u1b)
                u1s = s["u1s"] = wk.tile([128, 2, ETILE], EDT, tag="u1s",
                                         name=f"u1s{t}")
                for mc in range(2):
                    p = ps.tile([128, ETILE], F32, tag="mm", bufs=5)
                    mm_edge(p, [("u1f", mc, fT), ("u1l", mc, lT),
                                ("u1a", mc, aT)])
                    nc.scalar.activation(u1s[:, mc], p[:], ACTF.Silu,
                                         bias=B("u1b", mc), scale=IS)

            def stageA2(t):
                s = st[t]
                in8b, u1s = s["in8"], s["u1s"]
                fT = in8b[:, 0]

                # y = (ef@We + be) + sigmoid(gate)*(u1s@W2 + b2)
                y = s["y"] = wk.tile([128, 2, ETILE], BF16, tag="y",
                                     name=f"y{t}", bufs=3)
                for mc in range(2):
                    pg = ps.tile([128, ETILE], F32, tag="mm", bufs=5)
                    mm_edge(pg, [("gf", mc, fT), ("gu", mc, u1s)])
                    gate = wk.tile([128, ETILE], BF16, tag="gate")
                    nc.scalar.activation(gate[:], pg[:], ACTF.Sigmoid,
                                         bias=B("gb", mc), scale=IS)
                    pu = ps.tile([128, ETILE], F32, tag="mm", bufs=5)
                    mm_edge(pu, [("w2", mc, u1s)])
                    upd = wk.tile([128, ETILE], BF16, tag="upd")
                    nc.vector.tensor_scalar(upd[:], pu[:], IS, B("b2", mc),
                                            ALU.mult, ALU.add)
                    pe_ = ps.tile([128, ETILE], F32, tag="mm", bufs=5)
                    mm_edge(pe_, [("we", mc, fT)])
                    efc = wk.tile([128, ETILE], BF16, tag="efc")
                    nc.vector.tensor_scalar(efc[:], pe_[:], IS, B("be", mc),
                                            ALU.mult, ALU.add)
                    t0 = wk.tile([128, ETILE], BF16, tag="t0")
                    nc.vector.tensor_tensor(t0[:], gate[:], upd[:], ALU.mult)
                    nc.vector.tensor_tensor(y[:, mc], t0[:], efc[:], ALU.add)

                # LN stats: per-edge sums of y and y^2 over features via PE
                y2 = wk.tile([128, 2, ETILE], BF16, tag="y2")
                nc.vector.tensor_tensor(y2[:, 0], y[:, 0], y[:, 0], ALU.mult)
                nc.vector.tensor_tensor(y2[:, 1], y[:, 1], y[:, 1], ALU.mult)
                s1 = s["s1"] = ps.tile([1, ETILE], F32, tag="ln", bufs=2,
                                       name="s1")
                for c in range(2):
                    nc.tensor.matmul(s1[:], ones_cb[:], y[:, c],
                                     start=(c == 0), stop=(c == 1))
                s2 = s["s2"] = ps.tile([1, ETILE], F32, tag="ln", bufs=2,
                                       name="s2")
                for c in range(2):
                    nc.tensor.matmul(s2[:], ones_cb[:], y2[:, c],
                                     start=(c == 0), stop=(c == 1))
                rowsA = s["rowsA"] = wk.tile([1, ETILE], F32, tag="rowsA",
                                             name=f"rowsA{t}")
                nc.scalar.activation(rowsA[:], s1[:], ACTF.Copy)
                rowsB = s["rowsB"] = wk.tile([1, ETILE], F32, tag="rowsB",
                                             name=f"rowsB{t}")
                nc.scalar.activation(rowsB[:], s2[:], ACTF.Copy)

            def stageB(t):
                s = st[t]
                rowsA, rowsB = s["rowsA"], s["rowsB"]
                cfp = ps.tile([128, 4, 2], F32, tag="bc", bufs=1)
                for q in range(4):
                    nc.tensor.transpose(cfp[:, q, 0:1],
                                        rowsA[:, q * 128:(q + 1) * 128],
                                        identf_t[0:1, 0:1])
                    nc.tensor.transpose(cfp[:, q, 1:2],
                                        rowsB[:, q * 128:(q + 1) * 128],
                                        identf_t[0:1, 0:1])
                cfs = wk.tile([128, 4, 2], F32, tag="cfs")
                nc.vector.tensor_copy(cfs[:], cfp[:])
                # colform mean/var math + quake rsqrt (all [128,4], DVE only)
                mcol = wk.tile([128, 4], F32, tag="mcol")
                t1 = wk.tile([128, 4], F32, tag="t1c")
                ve = wk.tile([128, 4], F32, tag="ve")
                nc.vector.tensor_scalar_mul(mcol[:], cfs[:, :, 0], 1.0 / H)
                nc.vector.tensor_tensor(t1[:], cfs[:, :, 0], mcol[:], ALU.mult)
                nc.vector.tensor_tensor(t1[:], cfs[:, :, 1], t1[:],
                                        ALU.subtract)
                nc.vector.tensor_scalar(ve[:], t1[:], 1.0 / H, 1e-5, ALU.mult,
                                        ALU.add)
                x0 = wk.tile([128, 4], F32, tag="x0")
                x0u = x0[:].bitcast(U32)
                veu = ve[:].bitcast(U32)
                nc.vector.tensor_scalar(x0u, veu, 1, None,
                                        ALU.logical_shift_right)
                nc.vector.tensor_tensor(x0u, magic_t[:, 0:4], x0u,
                                        ALU.subtract)
                nw = wk.tile([128, 4], F32, tag="nw")
                for _ in range(1):
                    nc.vector.tensor_tensor(nw[:], x0[:], x0[:], ALU.mult)
                    nc.vector.tensor_tensor(nw[:], nw[:], ve[:], ALU.mult)
                    nc.vector.tensor_scalar(nw[:], nw[:], -0.5, 1.5, ALU.mult,
                                            ALU.add)
                    nc.vector.tensor_tensor(x0[:], x0[:], nw[:], ALU.mult)
                invn = wk.tile([128, 4, 2], BF16, tag="invn")
                nc.vector.tensor_copy(invn[:, :, 0], x0[:])
                nc.vector.tensor_tensor(invn[:, :, 1], mcol[:], x0[:],
                                        ALU.mult)
                r2i = ps.tile([1, ETILE], BF16, tag="bc", bufs=1)
                for q in range(4):
                    nc.tensor.transpose(r2i[:, q * 128:(q + 1) * 128],
                                        invn[:, q, 0:1], identb_t[:])
                rows2i = wk.tile([1, ETILE], BF16, tag="rows2i")
                nc.scalar.activation(rows2i[:], r2i[:], ACTF.Copy)
                r2n = ps.tile([1, ETILE], BF16, tag="bc", bufs=1)
                for q in range(4):
                    nc.tensor.transpose(r2n[:, q * 128:(q + 1) * 128],
                                        invn[:, q, 1:2], identb_t[:])
                rows2n = wk.tile([1, ETILE], BF16, tag="rows2n")
                nc.scalar.activation(rows2n[:], r2n[:], ACTF.Copy)
                bcp = ps.tile([128, ETILE], F32, tag="bc", bufs=1)
                nc.tensor.matmul(bcp[:], ones_rb[:], rows2i[:],
                                 start=True, stop=True)
                inv_bc = s["inv_bc"] = wk.tile([128, ETILE], BF16, tag="invbc",
                                               name=f"invbc{t}")
                nc.scalar.activation(inv_bc[:], bcp[:], ACTF.Copy)
                bcp2 = ps.tile([128, ETILE], F32, tag="bc", bufs=1)
                nc.tensor.matmul(bcp2[:], ones_rb[:], rows2n[:],
                                 start=True, stop=True)
                n_bc = s["n_bc"] = wk.tile([128, ETILE], BF16, tag="nbc",
                                           name=f"nbc{t}")
                nc.scalar.activation(n_bc[:], bcp2[:], ACTF.Copy)

            def stageC1(t):
                s = st[t]
                in16b, y = s["in16"], s["y"]
                qfT, qmT = in16b[:, 0], in16b[:, 1]
                inv_bc, n_bc = s["inv_bc"], s["n_bc"]

                eoT = s["eoT"] = wk.tile([128, 2, ETILE], BF16, tag="eoT",
                                         name=f"eoT{t}")
                for c in range(2):
                    d = wk.tile([128, ETILE], BF16, tag="d")
                    nc.vector.tensor_tensor(d[:], y[:, c], inv_bc[:], ALU.mult)
                    d2 = wk.tile([128, ETILE], BF16, tag="d2")
                    nc.vector.tensor_tensor(d2[:], d[:], n_bc[:], ALU.subtract)
                    d3 = wk.tile([128, ETILE], BF16, tag="d3")
                    nc.vector.tensor_scalar(d3[:], d2[:], B("lng", c),
                                            B("lnb", c), ALU.mult, ALU.add)
                    nc.vector.tensor_scalar_max(eoT[:, c], d3[:], 0.0)

                # message MLPs (qf/qm folded in via identity matmul)
                h1f = s["h1f"] = wk.tile([128, 2, ETILE], BF16, tag="h1f",
                                         name=f"h1f{t}")
                h1m = s["h1m"] = wk.tile([128, 2, ETILE], BF16, tag="h1m",
                                         name=f"h1m{t}")
                for mc in range(2):
                    p = ps.tile([128, ETILE], F32, tag="mm", bufs=5)
                    mm(p, [("f1c", kc, mc, eoT[:, kc]) for kc in range(2)],
                       tail=(identb_t[:], qfT[:, mc]))
                    nc.scalar.activation(h1f[:, mc], p[:], ACTF.Silu,
                                         bias=B("bf1", mc))
                    p2 = ps.tile([128, ETILE], F32, tag="mm", bufs=5)
                    mm(p2, [("m1c", kc, mc, eoT[:, kc]) for kc in range(2)],
                       tail=(identb_t[:], qmT[:, mc]))
                    nc.scalar.activation(h1m[:, mc], p2[:], ACTF.Silu,
                                         bias=B("bm1", mc))

            def stageC2(t):
                s = st.pop(t)
                h1f, h1m = s["h1f"], s["h1m"]

                score = wk.tile([128, 2, ETILE], BF16, tag="score")
                mbv = wk.tile([128, 2, ETILE], BF16, tag="mbv")
                msgT = wk.tile([128, 2, ETILE], BF16, tag="msgT")
                for mc in range(2):
                    pz = ps.tile([128, ETILE], F32, tag="mm", bufs=5)
                    mm(pz, [("f2", kc, mc, h1f[:, kc]) for kc in range(2)])
                    nc.scalar.activation(score[:, mc], pz[:], ACTF.Sigmoid,
                                         bias=B("Bs", mc), scale=B("As", mc))
                    pm = ps.tile([128, ETILE], F32, tag="mm", bufs=5)
                    mm(pm, [("m2", kc, mc, h1m[:, kc]) for kc in range(2)])
                    nc.vector.tensor_scalar_add(mbv[:, mc], pm[:], B("bm2", mc))
                    nc.vector.tensor_tensor(msgT[:, mc], score[:, mc],
                                            mbv[:, mc], ALU.mult)

                # transpose msg to edge-major (env is folded into the
                # host-built one-hot masks)
                msg_em = wk.tile([128, 4, H], BF16, tag="msg_em")
                for q in range(4):
                    tp = ps.tile([128, 2 * 128], BF16, tag="mm", bufs=5)
                    for c in range(2):
                        nc.tensor.transpose(
                            tp[:, c * 128:(c + 1) * 128],
                            msgT[:, c, q * 128:(q + 1) * 128], identb_t[:])
                    nc.vector.tensor_copy(msg_em[:, q], tp[:])
                ohb = s["oh"]
                b0 = base[t]
                for c in range(2):
                    psc = ps.tile([128, W], F32, tag="mm", bufs=5)
                    for q in range(4):
                        nc.tensor.matmul(
                            psc[:], msg_em[:, q, c * 128:(c + 1) * 128],
                            ohb[:, q], start=(q == 0), stop=(q == 3))
                    nc.vector.tensor_tensor(
                        agg[c][:, b0:b0 + W], agg[c][:, b0:b0 + W], psc[:],
                        ALU.add)

            DEPTH = 2
            for k in range(NT):
                stageA1(k)
                if k >= DEPTH:
                    stageC1(k - DEPTH)
                stageA2(k)
                if k >= DEPTH:
                    stageC2(k - DEPTH)
                if k >= 1:
                    stageB(k - 1)
            if NT >= 1:
                stageB(NT - 1)
            for k in range(max(0, NT - DEPTH), NT):
                stageC1(k)
                stageC2(k)

            # ============== BN-out stats allreduce + final ==============
            nc.sync.dma_start(xT_t[:, 0, :], xT_d[0:128, :])
            nc.sync.dma_start(xT_t[:, 1, :], xT_d[128:256, :])
            ast = cp.tile([128, 4], F32)
            scr2 = wk.tile([128, NLOC], F32, tag="scr2")
            scr3 = wk.tile([128, NLOC], F32, tag="scr3")
            nc.gpsimd.tensor_tensor(scr2[:], agg[0][:], agg[0][:], ALU.mult)
            nc.gpsimd.tensor_tensor(scr3[:], agg[1][:], agg[1][:], ALU.mult)
            nc.vector.tensor_reduce(
                ast[:, 0:1], agg[0][:], mybir.AxisListType.X, ALU.add)
            nc.vector.tensor_reduce(
                ast[:, 1:2], agg[1][:], mybir.AxisListType.X, ALU.add)
            nc.vector.tensor_reduce(
                ast[:, 2:3], scr2[:], mybir.AxisListType.X, ALU.add)
            nc.vector.tensor_reduce(
                ast[:, 3:4], scr3[:], mybir.AxisListType.X, ALU.add)
            nc.sync.dma_start(ccB_in[:], ast[:])
            nc.gpsimd.collective_compute(
                "AllReduce", ALU.add, ins=[ccB_in[:]], outs=[ccB_out[:]],
                replica_groups=RG)
            gB = cp.tile([128, 4], F32)
            nc.sync.dma_start(gB[:], ccB_out[:])
            eps_t = cp.tile([128, 1], F32)
            nc.vector.memset(eps_t[:], 1e-5)
            mO = cp.tile([128, 2], F32)
            nc.vector.tensor_scalar_mul(mO[:], gB[:, 0:2], 1.0 / N)
            vO = cp.tile([128, 2], F32)
            nc.vector.tensor_scalar_mul(vO[:], gB[:, 2:4], 1.0 / N)
            msqO = cp.tile([128, 2], F32)
            nc.vector.tensor_tensor(msqO[:], mO[:], mO[:], ALU.mult)
            nc.vector.tensor_tensor(vO[:], vO[:], msqO[:], ALU.subtract)
            nc.vector.tensor_scalar_add(vO[:], vO[:], 1e-5)
            # rsqrt via the same bit trick (tiny)
            iO = cp.tile([128, 2], F32)
            iOu = iO[:].bitcast(U32)
            vOu = vO[:].bitcast(U32)
            nc.vector.tensor_scalar(iOu, vOu, 1, None, ALU.logical_shift_right)
            nc.vector.tensor_tensor(iOu, magic_t[:, 0:2], iOu, ALU.subtract)
            nwO = cp.tile([128, 2], F32)
            for _ in range(2):
                nc.vector.tensor_tensor(nwO[:], iO[:], iO[:], ALU.mult)
                nc.vector.tensor_tensor(nwO[:], nwO[:], vO[:], ALU.mult)
                nc.vector.tensor_scalar(nwO[:], nwO[:], -0.5, 1.5, ALU.mult,
                                        ALU.add)
                nc.vector.tensor_tensor(iO[:], iO[:], nwO[:], ALU.mult)
            A2 = cp.tile([128, 2], F32)
            i_g = BIAS_ORDER.index("bnog")
            i_b = BIAS_ORDER.index("bnob")
            nc.vector.tensor_tensor(A2[:], iO[:],
                                    bias_t[:, 2 * i_g:2 * i_g + 2], ALU.mult)
            B2 = cp.tile([128, 2], F32)
            nc.vector.tensor_tensor(B2[:], mO[:], A2[:], ALU.mult)
            nc.vector.tensor_tensor(B2[:], bias_t[:, 2 * i_b:2 * i_b + 2],
                                    B2[:], ALU.subtract)

            for c in range(2):
                ot = wk.tile([128, NLOC], F32, tag="ot")
                nc.vector.tensor_scalar(
                    ot[:], agg[c][:], A2[:, c:c + 1], B2[:, c:c + 1],
                    ALU.mult, ALU.add)
                nc.vector.tensor_tensor(
                    ot[:], ot[:], xT_t[:, c, :], ALU.add)
                nc.vector.tensor_scalar_max(ot[:], ot[:], 0.0)
                nc.sync.dma_start(out_d[c * 128:(c + 1) * 128, :], ot[:])

    return nc


# ---------------------------------------------------------------------------

_CACHE = {}


def _get_program(cfg):
    key = tuple(sorted((k, v) for k, v in cfg.items()))
    if key not in _CACHE:
        _CACHE[key] = _build_program(cfg)
    return _CACHE[key]


def _assemble(cfg, results):
    N, NLOC = cfg["N"], cfg["NLOC"]
    out = np.empty((N, H), np.float32)
    for c in range(NCORES):
        out[c * NLOC:(c + 1) * NLOC] = results[c]["out"].T
    return out


def kernel(**inputs):
    cfg, in_maps = _prepare(inputs)
    nc = _get_program(cfg)
    res = run_bass_kernel_spmd(nc, in_maps, list(range(NCORES)))
    return _assemble(cfg, res.results)
